# revision 6
# baseline (speedup 1.0000x reference)
"""Block-diagonal linear (grouped GEMM) on 8 TRN2 NeuronCores.

out[b, g*512+n] = sum_k x[b, g*512+k] * blocks[g, k, n]

Sharding: group-parallel — core g computes x[:, g*512:(g+1)*512] @ blocks[g].
Per-core kernel: for each 128-token tile, PE-transpose the x tile (fp32,
exact), round to float32r in the PSUM->SBUF copy, then run the K=512
accumulation as 4 float32r matmuls (full-rate on the PE at N=512).
"""
import numpy as np

import concourse.bacc as bacc
import concourse.tile as tile
from concourse import masks, mybir
from concourse.bass_utils import run_bass_kernel_spmd

TOKENS = 8192
G = 8
M = 512  # per-block in-features
N = 512  # per-block out-features
P = 128
MT = TOKENS // P  # 64 token tiles
KT = M // P       # 4 contraction tiles
F32 = mybir.dt.float32
F32R = mybir.dt.float32r

_CACHE: dict = {}


def _batches():
    """Variable m-tile batch schedule: small head/tail for ramp, 4-tile steady."""
    sched = [1, 1, 2] + [4] * 14 + [2, 1, 1]
    assert sum(sched) == MT
    start = 0
    for n in sched:
        yield start, n
        start += n


def _body(tc, nc, x, w, out):
    x_v = x.rearrange("(m p) k -> m p k", p=P)      # [64, 128, 512]
    out_v = out.rearrange("(m p) n -> m p n", p=P)  # [64, 128, 512]
    with (
        tc.tile_pool(name="const", bufs=1) as constp,
        tc.tile_pool(name="wp", bufs=1) as wp,
        tc.tile_pool(name="xin", bufs=4) as xin,
        tc.tile_pool(name="xtr", bufs=6) as xtr,
        tc.tile_pool(name="outp", bufs=6) as outp,
        tc.tile_pool(name="psx", bufs=4, space="PSUM") as psx,
        tc.tile_pool(name="pso", bufs=4, space="PSUM") as pso,
    ):
        ident = constp.tile([P, P], F32)
        masks.make_identity(nc, ident[:])

        # weights: [512, 512] -> [128, kt, 512], rounded once to f32r.
        # Loaded on the gpsimd (SWDGE) ring so x transfers start immediately
        # on the sync (HWDGE) ring.
        w_f = wp.tile([P, KT, N], F32, tag="wf")
        nc.gpsimd.dma_start(w_f[:], w.rearrange("(j p) n -> p j n", p=P))
        w_r = wp.tile([P, KT, N], F32R, tag="wr")
        nc.vector.tensor_copy(w_r[:], w_f[:])

        OB = 2  # m-tiles per output flush
        o_t = None
        for m0, nb in _batches():
            x_t = xin.tile([P, 4, M], F32, tag="x")
            nc.sync.dma_start(
                x_t[:, :nb, :],
                x_v[m0:m0 + nb].rearrange("b p k -> p b k"),
            )
            for b in range(nb):
                mt = m0 + b
                # transpose x tile: 4 blocks into one PSUM bank
                ps_xT = psx.tile([P, M], F32, tag="psx")
                for j in range(KT):
                    nc.tensor.matmul(
                        ps_xT[:, j * P:(j + 1) * P],
                        x_t[:, b, j * P:(j + 1) * P],
                        ident[:],
                        is_transpose=True,
                        start=(j == 0),
                        stop=(j == KT - 1),
                    )
                xT_r = xtr.tile([P, M], F32R, tag="xT")
                nc.vector.tensor_copy(xT_r[:], ps_xT[:])

                ps_o = pso.tile([P, N], F32, tag="pso")
                for j in range(KT):
                    nc.tensor.matmul(
                        ps_o[:],
                        xT_r[:, j * P:(j + 1) * P],
                        w_r[:, j, :],
                        start=(j == 0),
                        stop=(j == KT - 1),
                    )
                if mt % OB == 0:
                    o_t = outp.tile([P, OB, N], F32, tag="o")
                nc.scalar.copy(o_t[:, mt % OB, :], ps_o[:])
                if mt % OB == OB - 1:
                    nc.gpsimd.dma_start(
                        out_v[mt - OB + 1:mt + 1].rearrange("b p n -> p b n"),
                        o_t[:],
                    )


def _build():
    nc = bacc.Bacc("TRN2", target_bir_lowering=False, debug=False, num_devices=G)
    x = nc.dram_tensor("x", [TOKENS, M], F32, kind="ExternalInput").ap()
    w = nc.dram_tensor("w", [M, N], F32, kind="ExternalInput").ap()
    out = nc.dram_tensor("out", [TOKENS, N], F32, kind="ExternalOutput").ap()
    with tile.TileContext(nc) as tc:
        _body(tc, nc, x, w, out)
    nc.compile()
    return nc


def _run(in_maps, **kwargs):
    if "nc" not in _CACHE:
        _CACHE["nc"] = _build()
    return run_bass_kernel_spmd(_CACHE["nc"], in_maps, list(range(G)), **kwargs)


def _in_maps(x, blocks):
    return [
        {
            "x": np.ascontiguousarray(x[:, g * M:(g + 1) * M], dtype=np.float32),
            "w": np.ascontiguousarray(blocks[g], dtype=np.float32),
        }
        for g in range(G)
    ]


def kernel(x, blocks):
    x = np.asarray(x)
    blocks = np.asarray(blocks)
    res = _run(_in_maps(x, blocks))
    return np.concatenate([res.results[g]["out"] for g in range(G)], axis=1)


# revision 8
# speedup vs baseline: 1.0264x; 1.0264x over previous
"""Block-diagonal linear (grouped GEMM) on 8 TRN2 NeuronCores.

out[b, g*512+n] = sum_k x[b, g*512+k] * blocks[g, k, n]

Sharding: group-parallel — core g computes x[:, g*512:(g+1)*512] @ blocks[g].
Per-core kernel: for each 128-token tile, PE-transpose the x tile (fp32,
exact), round to float32r in the PSUM->SBUF copy, then run the K=512
accumulation as 4 float32r matmuls (full-rate on the PE at N=512).
"""
import numpy as np

import concourse.bacc as bacc
import concourse.tile as tile
from concourse import masks, mybir
from concourse.bass_utils import run_bass_kernel_spmd

TOKENS = 8192
G = 8
M = 512  # per-block in-features
N = 512  # per-block out-features
P = 128
MT = TOKENS // P  # 64 token tiles
KT = M // P       # 4 contraction tiles
F32 = mybir.dt.float32
F32R = mybir.dt.float32r

_CACHE: dict = {}


def _batches():
    """Variable m-tile batch schedule: small head/tail for ramp, 4-tile steady."""
    sched = [1, 1, 2] + [4] * 14 + [2, 1, 1]
    assert sum(sched) == MT
    start = 0
    for n in sched:
        yield start, n
        start += n


def _body(tc, nc, x, w, out):
    x_v = x.rearrange("(m p) k -> m p k", p=P)      # [64, 128, 512]
    out_v = out.rearrange("(m p) n -> m p n", p=P)  # [64, 128, 512]
    with (
        tc.tile_pool(name="const", bufs=1) as constp,
        tc.tile_pool(name="wp", bufs=1) as wp,
        tc.tile_pool(name="xin", bufs=4) as xin,
        tc.tile_pool(name="xtr", bufs=6) as xtr,
        tc.tile_pool(name="outp", bufs=6) as outp,
        tc.tile_pool(name="psx", bufs=4, space="PSUM") as psx,
        tc.tile_pool(name="pso", bufs=4, space="PSUM") as pso,
    ):
        ident = constp.tile([P, P], F32)
        masks.make_identity(nc, ident[:])

        # weights: [512, 512] -> [128, kt, 512], rounded once to f32r.
        # k-tile j is loaded right after the j-th early x batch so the first
        # x tile transfer leads the ring and W trickles in behind it.
        w_f = wp.tile([P, KT, N], F32, tag="wf")
        w_r = wp.tile([P, KT, N], F32R, tag="wr")
        w_v = w.rearrange("(j p) n -> j p n", p=P)

        OB = 4  # m-tiles per output flush
        o_t = None
        for bi, (m0, nb) in enumerate(_batches()):
            x_t = xin.tile([P, 4, M], F32, tag="x")
            nc.sync.dma_start(
                x_t[:, :nb, :],
                x_v[m0:m0 + nb].rearrange("b p k -> p b k"),
            )
            if bi == 0:
                # W rides the ring right behind the first x tile
                for j in range(KT):
                    nc.sync.dma_start(w_f[:, j, :], w_v[j])
                    nc.vector.tensor_copy(w_r[:, j, :], w_f[:, j, :])
            for b in range(nb):
                mt = m0 + b
                # transpose x tile: 4 blocks into one PSUM bank
                ps_xT = psx.tile([P, M], F32, tag="psx")
                for j in range(KT):
                    nc.tensor.matmul(
                        ps_xT[:, j * P:(j + 1) * P],
                        x_t[:, b, j * P:(j + 1) * P],
                        ident[:],
                        is_transpose=True,
                        start=(j == 0),
                        stop=(j == KT - 1),
                    )
                xT_r = xtr.tile([P, M], F32R, tag="xT")
                nc.vector.tensor_copy(xT_r[:], ps_xT[:])

                ps_o = pso.tile([P, N], F32, tag="pso")
                for j in range(KT):
                    nc.tensor.matmul(
                        ps_o[:],
                        xT_r[:, j * P:(j + 1) * P],
                        w_r[:, j, :],
                        start=(j == 0),
                        stop=(j == KT - 1),
                    )
                if mt % OB == 0:
                    o_t = outp.tile([P, OB, N], F32, tag="o")
                nc.scalar.copy(o_t[:, mt % OB, :], ps_o[:])
                if mt % OB == OB - 1:
                    # tail outputs ride the sync ring (input is done by then)
                    eng = nc.sync if mt >= MT - 2 * OB else nc.gpsimd
                    eng.dma_start(
                        out_v[mt - OB + 1:mt + 1].rearrange("b p n -> p b n"),
                        o_t[:],
                    )


def _build():
    nc = bacc.Bacc("TRN2", target_bir_lowering=False, debug=False, num_devices=G)
    x = nc.dram_tensor("x", [TOKENS, M], F32, kind="ExternalInput").ap()
    w = nc.dram_tensor("w", [M, N], F32, kind="ExternalInput").ap()
    out = nc.dram_tensor("out", [TOKENS, N], F32, kind="ExternalOutput").ap()
    with tile.TileContext(nc) as tc:
        _body(tc, nc, x, w, out)
    nc.compile()
    return nc


def _run(in_maps, **kwargs):
    if "nc" not in _CACHE:
        _CACHE["nc"] = _build()
    return run_bass_kernel_spmd(_CACHE["nc"], in_maps, list(range(G)), **kwargs)


def _in_maps(x, blocks):
    return [
        {
            "x": np.ascontiguousarray(x[:, g * M:(g + 1) * M], dtype=np.float32),
            "w": np.ascontiguousarray(blocks[g], dtype=np.float32),
        }
        for g in range(G)
    ]


def kernel(x, blocks):
    x = np.asarray(x)
    blocks = np.asarray(blocks)
    res = _run(_in_maps(x, blocks))
    return np.concatenate([res.results[g]["out"] for g in range(G)], axis=1)


# revision 14
# speedup vs baseline: 1.0925x; 1.0644x over previous
"""Block-diagonal linear (grouped GEMM) on 8 TRN2 NeuronCores.

out[b, g*512+n] = sum_k x[b, g*512+k] * blocks[g, k, n]

Sharding: group-parallel — core g computes x[:, g*512:(g+1)*512] @ blocks[g].
Per-core kernel: for each 128-token tile, PE-transpose the x tile (fp32,
exact), round to float32r in the PSUM->SBUF copy, then run the K=512
accumulation as 4 float32r matmuls (full-rate on the PE at N=512).
"""
import numpy as np

import concourse.bacc as bacc
import concourse.tile as tile
from concourse import masks, mybir
from concourse.bass_utils import run_bass_kernel_spmd

TOKENS = 8192
G = 8
M = 512  # per-block in-features
N = 512  # per-block out-features
P = 128
MT = TOKENS // P  # 64 token tiles
KT = M // P       # 4 contraction tiles
F32 = mybir.dt.float32
F32R = mybir.dt.float32r

_CACHE: dict = {}


def _batches():
    """Variable m-tile batch schedule: small head/tail for ramp, 4-tile steady."""
    sched = [1, 1, 2] + [4] * 14 + [2, 1, 1]
    assert sum(sched) == MT
    start = 0
    for n in sched:
        yield start, n
        start += n


def _body(tc, nc, x, w, out):
    x_v = x.rearrange("(m p) k -> m p k", p=P)      # [64, 128, 512]
    out_v = out.rearrange("(m p) n -> m p n", p=P)  # [64, 128, 512]
    with (
        tc.tile_pool(name="const", bufs=1) as constp,
        tc.tile_pool(name="wp", bufs=1) as wp,
        tc.tile_pool(name="xin", bufs=4) as xin,
        tc.tile_pool(name="xtr", bufs=6) as xtr,
        tc.tile_pool(name="outp", bufs=6) as outp,
        tc.tile_pool(name="psx", bufs=4, space="PSUM") as psx,
        tc.tile_pool(name="pso", bufs=4, space="PSUM") as pso,
    ):
        # f32r identity: transpose streaming at 1.5 cyc/row vs 2.0 for fp32;
        # values are 0/1 so precision is exact. memset can't write f32r, so
        # build in fp32 and round-copy once.
        ident_f = constp.tile([P, P], F32, tag="idf")
        masks.make_identity(nc, ident_f[:])
        ident = constp.tile([P, P], F32R, tag="idr")
        nc.vector.tensor_copy(ident[:], ident_f[:])

        # weights: [512, 512] -> [128, kt, 512], rounded once to f32r.
        # k-tile j is loaded right after the j-th early x batch so the first
        # x tile transfer leads the ring and W trickles in behind it.
        w_f = wp.tile([P, KT, N], F32, tag="wf")
        w_r = wp.tile([P, KT, N], F32R, tag="wr")
        w_v = w.rearrange("(j p) n -> j p n", p=P)

        OB = 4  # m-tiles per output flush
        o_t = None
        for bi, (m0, nb) in enumerate(_batches()):
            x_t = xin.tile([P, 4, M], F32R, tag="x")
            nc.sync.dma_start(
                x_t[:, :nb, :],
                x_v[m0:m0 + nb].rearrange("b p k -> p b k").bitcast(F32R),
            )
            if bi == 0:
                # W rides the ring right behind the first x tile
                for j in range(KT):
                    nc.sync.dma_start(w_f[:, j, :], w_v[j])
                    nc.vector.tensor_copy(w_r[:, j, :], w_f[:, j, :])
            for b in range(nb):
                mt = m0 + b
                # transpose x tile: 4 blocks into one PSUM bank
                ps_xT = psx.tile([P, M], F32R, tag="psx")
                for j in range(KT):
                    nc.tensor.matmul(
                        ps_xT[:, j * P:(j + 1) * P],
                        x_t[:, b, j * P:(j + 1) * P],
                        ident[:],
                        is_transpose=True,
                        start=(j == 0),
                        stop=(j == KT - 1),
                    )
                xT_r = xtr.tile([P, M], F32R, tag="xT")
                nc.vector.tensor_copy(xT_r[:], ps_xT[:])

                ps_o = pso.tile([P, N], F32, tag="pso")
                for j in range(KT):
                    nc.tensor.matmul(
                        ps_o[:],
                        xT_r[:, j * P:(j + 1) * P],
                        w_r[:, j, :],
                        start=(j == 0),
                        stop=(j == KT - 1),
                    )
                ob = mt % OB
                if ob == 0:
                    o_t = outp.tile([P, OB, N], F32, tag="o")
                nc.scalar.copy(o_t[:, ob, :], ps_o[:])
                # steady state: flush 4 m-tiles per SWDGE DMA; tail: flush
                # each of the last 4 m-tiles individually on the sync ring
                # (input traffic is done by then) so the drain is short.
                if mt >= MT - OB:
                    nc.sync.dma_start(out_v[mt].rearrange("p n -> p () n"),
                                      o_t[:, ob:ob + 1, :])
                elif ob == OB - 1:
                    nc.gpsimd.dma_start(
                        out_v[mt - OB + 1:mt + 1].rearrange("b p n -> p b n"),
                        o_t[:],
                    )


def _build():
    nc = bacc.Bacc("TRN2", target_bir_lowering=False, debug=False, num_devices=G)
    x = nc.dram_tensor("x", [TOKENS, M], F32, kind="ExternalInput").ap()
    w = nc.dram_tensor("w", [M, N], F32, kind="ExternalInput").ap()
    out = nc.dram_tensor("out", [TOKENS, N], F32, kind="ExternalOutput").ap()
    with tile.TileContext(nc) as tc:
        _body(tc, nc, x, w, out)
    nc.compile()
    return nc


def _run(in_maps, **kwargs):
    if "nc" not in _CACHE:
        _CACHE["nc"] = _build()
    return run_bass_kernel_spmd(_CACHE["nc"], in_maps, list(range(G)), **kwargs)


def _in_maps(x, blocks):
    return [
        {
            "x": np.ascontiguousarray(x[:, g * M:(g + 1) * M], dtype=np.float32),
            "w": np.ascontiguousarray(blocks[g], dtype=np.float32),
        }
        for g in range(G)
    ]


def kernel(x, blocks):
    x = np.asarray(x)
    blocks = np.asarray(blocks)
    res = _run(_in_maps(x, blocks))
    return np.concatenate([res.results[g]["out"] for g in range(G)], axis=1)


# revision 17
# speedup vs baseline: 1.1016x; 1.0083x over previous
"""Block-diagonal linear (grouped GEMM) on 8 TRN2 NeuronCores.

out[b, g*512+n] = sum_k x[b, g*512+k] * blocks[g, k, n]

Sharding: group-parallel — core g computes block g's GEMM. The host hands
each core xT = x[:, g*512:(g+1)*512].T ([512, 8192], feature-major) and
receives outT ([512, 8192]); the transposes happen on the host so the
device needs no PE transposes and every DMA stream reads/writes long
contiguous runs per partition.

Per-core kernel: out.T = W.T @ x.T as 64 PSUM accumulation groups:
psum[n-tile 128, m 512] += W[k-tile, n-tile].T @ xT[k-tile, m-chunk],
with all matmul operands rounded to float32r (full PE rate at N=512,
~1.5e-4 max rel err vs fp32).
"""
import numpy as np

import concourse.bacc as bacc
import concourse.tile as tile
from concourse import mybir
from concourse.bass_utils import run_bass_kernel_spmd

TOKENS = 8192
G = 8
M = 512  # per-block in-features
N = 512  # per-block out-features
P = 128
KT = M // P  # 4 contraction tiles
NT = N // P  # 4 output feature tiles
SUB = 512    # tokens per PSUM group (moving-dim max for 4-byte dtypes)
F32 = mybir.dt.float32
F32R = mybir.dt.float32r

# token-chunk schedule: small head/tail for pipeline ramp, 2048 steady
CHUNKS = [512, 512, 1024, 2048, 2048, 1024, 512, 512]
assert sum(CHUNKS) == TOKENS
CMAX = max(CHUNKS)

_CACHE: dict = {}


def _body(tc, nc, xT, w, outT):
    with (
        tc.tile_pool(name="wp", bufs=1) as wp,
        tc.tile_pool(name="xin", bufs=8) as xin,
        tc.tile_pool(name="outp", bufs=2) as outp,
        tc.tile_pool(name="pso", bufs=8, space="PSUM") as pso,
    ):
        # weights [512, 512] -> [128, kt, 512] fp32, rounded once to f32r
        w_f = wp.tile([P, KT, N], F32, tag="wf")
        w_r = wp.tile([P, KT, N], F32R, tag="wr")
        w_v = w.rearrange("(j p) n -> j p n", p=P)

        m0 = 0
        for ci, c in enumerate(CHUNKS):
            # load + round the 4 k-tiles of this token chunk
            xs = []
            for j in range(KT):
                x_t = xin.tile([P, CMAX], F32R, tag="x")
                nc.sync.dma_start(
                    x_t[:, :c], xT[j * P:(j + 1) * P, m0:m0 + c].bitcast(F32R)
                )
                # round to f32r in place (read+write same AP is per-element safe)
                nc.vector.tensor_copy(x_t[:, :c], x_t[:, :c])
                xs.append(x_t)
            if ci == 0:
                # W rides the ring right behind the first chunk
                for j in range(KT):
                    nc.sync.dma_start(w_f[:, j, :], w_v[j])
                    nc.vector.tensor_copy(w_r[:, j, :], w_f[:, j, :])

            ots = [outp.tile([P, CMAX], F32, tag=f"o{nt}", name=f"ot{nt}") for nt in range(NT)]
            for s in range(c // SUB):
                for nt in range(NT):
                    ps_o = pso.tile([P, SUB], F32, tag="pso")
                    for j in range(KT):
                        nc.tensor.matmul(
                            ps_o[:],
                            w_r[:, j, nt * P:(nt + 1) * P],
                            xs[j][:, s * SUB:(s + 1) * SUB],
                            start=(j == 0),
                            stop=(j == KT - 1),
                        )
                    nc.scalar.copy(ots[nt][:, s * SUB:(s + 1) * SUB], ps_o[:])
            # flush the chunk: one DMA per n-tile; tail chunks ride the
            # sync ring (input traffic is finished by then)
            eng = nc.sync if ci >= len(CHUNKS) - 2 else nc.gpsimd
            for nt in range(NT):
                eng.dma_start(outT[nt * P:(nt + 1) * P, m0:m0 + c], ots[nt][:, :c])
            m0 += c


def _build():
    nc = bacc.Bacc("TRN2", target_bir_lowering=False, debug=False, num_devices=G)
    xT = nc.dram_tensor("xT", [M, TOKENS], F32, kind="ExternalInput").ap()
    w = nc.dram_tensor("w", [M, N], F32, kind="ExternalInput").ap()
    outT = nc.dram_tensor("outT", [N, TOKENS], F32, kind="ExternalOutput").ap()
    with tile.TileContext(nc) as tc:
        _body(tc, nc, xT, w, outT)
    nc.compile()
    return nc


def _run(in_maps, **kwargs):
    if "nc" not in _CACHE:
        _CACHE["nc"] = _build()
    return run_bass_kernel_spmd(_CACHE["nc"], in_maps, list(range(G)), **kwargs)


def _in_maps(x, blocks):
    return [
        {
            "xT": np.ascontiguousarray(x[:, g * M:(g + 1) * M].T, dtype=np.float32),
            "w": np.ascontiguousarray(blocks[g], dtype=np.float32),
        }
        for g in range(G)
    ]


def kernel(x, blocks):
    x = np.asarray(x)
    blocks = np.asarray(blocks)
    res = _run(_in_maps(x, blocks))
    return np.concatenate(
        [res.results[g]["outT"].T for g in range(G)], axis=1
    ).astype(np.float32, copy=False)
